# revision 28
# baseline (speedup 1.0000x reference)
"""MultiHeadLinearAttention Trainium2 kernel (8-core SPMD, fp8 DoubleRow).

Sharding: 16384 tokens split across 8 cores (core c: batch c//2, sequence half
c%2). All projections/attention/out-proj are local; the only cross-core
dependency is the per-batch KV summary (kv+ksum packed [256,260]) reduced via a
pair-wise AllReduce.

Numerics (validated in numpy sim; rel err ~1e-2 vs 2e-2 gate):
  - x and all weights pre-quantized host-side to fp8e4m3; weights scaled x64
    (uniform(-1/32,1/32) would be subnormal in fp8 otherwise)
  - all 7 projections run as fp8 DoubleRow matmuls (K=256/instr, 0.5 cyc/row)
  - q,k,v GLUs single-pass fp8 (q/k errors cancel in the num/z ratio);
    the out projection uses weight-split error feedback (W ~ W_hi + W_lo)
  - GLU intermediates bf16 (DVE 2x/4x modes need 2-byte dtypes);
    kv/z/num matmuls bf16; r kept fp32; attn stored as fp8(64*attn)
  - biases enter PSUM via K=1 DoubleRow matmuls (token-major k/v) or ACT
    bias (feature-major q); out bias + 1/4096 unscale applied on host

Engine balance: PE ~170us is the roof; elementwise split DVE/ACT/Pool:
  - ACT: silu/exp only (exp batched by groups of 8 tiles / chunk pairs --
    silu and exp live in different act tables, each switch costs 1283ns)
  - DVE: GLU products (psum reads), phi assembly (bf16 ts/tt fast modes),
    attn eviction, half the out evictions
  - Pool/gpsimd: 1/z partition-broadcast (replaces PE broadcast matmuls),
    weight DMA issue, collective
"""
import os
from contextlib import ExitStack

import ml_dtypes
import numpy as np
import bass_rust
import concourse.bass as bass
import concourse.mybir as mybir
import concourse.tile as tile
from concourse import bacc
from concourse.bass_utils import run_bass_kernel_spmd

F32 = mybir.dt.float32
BF16 = mybir.dt.bfloat16
F8 = mybir.dt.float8e4
ACTF = mybir.ActivationFunctionType
ALU = mybir.AluOpType
DR = mybir.MatmulPerfMode.DoubleRow

B, S, D, H = 4, 4096, 1024, 16
DK = D // H          # 64
EPS = 1e-6
NCORES = 8
T = B * S // NCORES  # 2048 tokens per core
P = 128
NM = T // P          # 16 token tiles
NCD = D // P         # 8 d-chunks
CH = 256             # stage-2 token chunk
NCH = T // CH        # 8 chunks
GB = 8               # stage-1a exp batch group size
GROUPS = [[0, 1], [2, 3], [4, 5], [6, 7]]

V_SPLIT = False      # single-pass fp8 v GLU (stage-1 is PE-bound; error ~11.4e-3)
WO_SPLIT = True      # weight-split error feedback on the out projection

NP8 = ml_dtypes.float8_e4m3


def build(single_core=False):
    nc = bacc.Bacc("TRN2", target_bir_lowering=False, debug=False,
                   num_devices=1 if single_core else NCORES)
    dt_in = {}

    def inp(name, shape, dtype=F8):
        dt_in[name] = nc.dram_tensor(name, shape, dtype, kind="ExternalInput").ap()

    inp("xk8", [NM, P, NCD, P])
    inp("xv8", [NM, P, NCD, P])
    inp("xq8", [NCH, P, NCD, CH])
    for nm in ("wk1", "wk2", "wq1", "wq2"):
        inp(nm, [P, NCD, D])
    vws = ["wv1h", "wv2h"] + (["wv1l", "wv2l"] if V_SPLIT else [])
    for nm in vws:
        inp(nm, [P, NCD, D])
    ows = ["woh"] + (["wol"] if WO_SPLIT else [])
    for nm in ows:
        inp(nm, [P, NCD, D])
    for nm in ("bk1", "bk2", "bv1", "bv2"):
        inp(nm, [1, 2, D])
    inp("ones2", [1, 2, P])
    inp("bq1c", [P, NCD], F32)
    inp("bq2c64", [P, NCD], F32)
    inp("maskp", [P, NM], F32)
    out = nc.dram_tensor("out", [T, D], F32, kind="ExternalOutput").ap()

    with tile.TileContext(nc) as tc:
        _emit(nc, tc, dt_in, out, single_core)
    nc.compile()
    return nc


def _emit(nc, tc, dt, out, single_core):
    def mm(ps, l, r, start, stop, skip=False):
        nc.tensor.matmul(ps, l, r, start=start, stop=stop, perf_mode=DR,
                         skip_group_check=skip)

    # chain ACT instructions so the scheduler preserves emission order --
    # silu and exp live in different act-func tables (1283ns per switch), and
    # the list scheduler otherwise interleaves them freely
    last_act = [None]

    def act(out_ap, in_ap, func, **kw):
        inst = nc.scalar.activation(out_ap, in_ap, func, **kw)
        if last_act[0] is not None:
            bass_rust.add_dep_helper(inst.ins, last_act[0].ins,
                                     reason="act-table-order")
        last_act[0] = inst
        return inst

    with ExitStack() as st0:
        const = st0.enter_context(tc.tile_pool(name="const", bufs=1))
        dram = st0.enter_context(tc.tile_pool(name="dram", bufs=1, space="DRAM"))
        kvres = st0.enter_context(tc.tile_pool(name="kvres", bufs=1))

        ones2_sb = const.tile([1, 2, P], F8, name="ones2_sb")
        nc.scalar.dma_start(ones2_sb[:], dt["ones2"][:])
        bq1c_sb = const.tile([P, NCD], F32, name="bq1c_sb")
        nc.scalar.dma_start(bq1c_sb[:], dt["bq1c"][:])
        bq2c64_sb = const.tile([P, NCD], F32, name="bq2c64_sb")
        nc.scalar.dma_start(bq2c64_sb[:], dt["bq2c64"][:])
        maskp_sb = const.tile([P, NM], F32, name="maskp_sb")
        nc.scalar.dma_start(maskp_sb[:], dt["maskp"][:])
        ones16 = const.tile([P, H], BF16, name="ones16")
        nc.gpsimd.memset(ones16[:], 1.0)

        # stage-2 weight pool at st0 scope so DMAs can prefetch during stage 1b
        wqop = st0.enter_context(tc.tile_pool(name="wqop", bufs=1, side="right"))

        kvstage = st0.enter_context(tc.tile_pool(name="kvstage", bufs=1))

        st1 = st0.enter_context(ExitStack())
        phik_pool = st1.enter_context(tc.tile_pool(name="phik", bufs=1))
        phi_k = [phik_pool.tile([P, D], BF16, tag=f"phik_{m}", name=f"phik_{m}")
                 for m in range(NM)]

        # wv pool spans stage 1a (prefetch) + stage 1b (use)
        stv = ExitStack()
        wvp = stv.enter_context(tc.tile_pool(name="wv", bufs=1))

        # ================= stage 1a: k projection -> phi_k =================
        with ExitStack() as st1a:
            wkp = st1a.enter_context(tc.tile_pool(name="wk", bufs=1))
            xkp = st1a.enter_context(tc.tile_pool(name="xk", bufs=2))
            g1p = st1a.enter_context(tc.tile_pool(name="g1p", bufs=2))
            kgp = st1a.enter_context(tc.tile_pool(name="kgp", bufs=2))
            mintp = st1a.enter_context(tc.tile_pool(name="mintp", bufs=GB + 2))
            trelp = st1a.enter_context(tc.tile_pool(name="trelp", bufs=GB + 2))
            texpp = st1a.enter_context(tc.tile_pool(name="texpp", bufs=2))
            pk1 = st1a.enter_context(tc.tile_pool(name="pk1", bufs=4, space="PSUM"))
            pk2 = st1a.enter_context(tc.tile_pool(name="pk2", bufs=4, space="PSUM"))

            bk_sb = {}
            for w, src in ((0, "bk1"), (1, "bk2")):
                bk_sb[w] = wkp.tile([1, 2, D], F8, tag=f"bk{w}", name=f"bk{w}")
                nc.sync.dma_start(bk_sb[w][:], dt[src][:])
            xk0 = xkp.tile([P, NCD, P], F8, tag="xk", name="xk0")
            nc.sync.dma_start(xk0[:], dt["xk8"][0, :, :, :])
            wk_sb = {}
            for w, src in ((0, "wk1"), (1, "wk2")):
                wk_sb[w] = wkp.tile([P, NCD, D], F8, tag=f"wk{w}", name=f"wk{w}")
                for hf in range(2):
                    ns = slice(hf * 512, (hf + 1) * 512)
                    nc.sync.dma_start(wk_sb[w][:, :, ns], dt[src][:, :, ns])
            wv_sb = {}
            bv_sb = {}

            def wv_prefetch():
                vnames = [("1h", "wv1h"), ("2h", "wv2h")]
                if V_SPLIT:
                    vnames += [("1l", "wv1l"), ("2l", "wv2l")]
                for w, src in vnames:
                    wv_sb[w] = wvp.tile([P, NCD, D], F8, tag=f"wv{w}",
                                        name=f"wv{w}")
                    nc.scalar.dma_start(wv_sb[w][:], dt[src][:])
                for w, src in ((0, "bv1"), (1, "bv2")):
                    bv_sb[w] = wvp.tile([1, 2, D], F8, tag=f"bv{w}",
                                        name=f"bv{w}")
                    nc.scalar.dma_start(bv_sb[w][:], dt[src][:])

            mints, trels = {}, {}

            def phi_flush(ms):
                for m2 in ms:
                    texp = texpp.tile([P, D], BF16, tag="texp", name="texp")
                    act(texp[:], mints[m2][:], ACTF.Exp, scale=1.0 / 64)
                    nc.vector.tensor_tensor(phi_k[m2][:], texp[:], trels[m2][:],
                                            ALU.add)

            for m in range(NM):
                xk_m = xkp.tile([P, NCD, P], F8, tag="xk", name="xk")
                nc.sync.dma_start(xk_m[:], dt["xk8"][m, :, :, :])
                kg = kgp.tile([P, D], BF16, tag="kg", name="kg")
                g1 = g1p.tile([P, D], BF16, tag="g1", name="g1")
                for half in range(2):
                    ns = slice(half * 512, (half + 1) * 512)
                    p1 = pk1.tile([P, 512], F32, tag="p1", name="p1")
                    p2 = pk2.tile([P, 512], F32, tag="p2", name="p2")
                    mm(p1[:], ones2_sb[:], bk_sb[0][:, :, ns], True, False)
                    for cp in range(4):
                        mm(p1[:], xk_m[:, 2 * cp:2 * cp + 2, :],
                           wk_sb[0][:, 2 * cp:2 * cp + 2, ns], False, cp == 3)
                    mm(p2[:], ones2_sb[:], bk_sb[1][:, :, ns], True, False)
                    for cp in range(4):
                        mm(p2[:], xk_m[:, 2 * cp:2 * cp + 2, :],
                           wk_sb[1][:, 2 * cp:2 * cp + 2, ns], False, cp == 3)
                    act(g1[:, ns], p1[:], ACTF.Silu, scale=1.0 / 64)
                    nc.vector.tensor_tensor(kg[:, ns], g1[:, ns], p2[:], ALU.mult)
                mint = mintp.tile([P, D], BF16, tag="mint", name="mint")
                nc.vector.tensor_scalar_min(mint[:], kg[:], 0.0)
                trel = trelp.tile([P, D], BF16, tag="trel", name="trel")
                nc.vector.tensor_scalar(trel[:], kg[:], 0.0, 1.0 / 64,
                                        ALU.max, ALU.mult)
                mints[m], trels[m] = mint, trel
                if m == 1:
                    wv_prefetch()
                if m % GB == GB - 1:
                    phi_flush(range(m - GB + 1, m + 1))

        # prefetch stage-2 weights during stage 1b
        wq_sb = {}
        for w, src in ((0, "wq1"), (1, "wq2")):
            wq_sb[w] = wqop.tile([P, NCD, D], F8, tag=f"wq{w}", name=f"wq{w}")
            nc.sync.dma_start(wq_sb[w][:], dt[src][:])
        wo_sb = []
        for src in (["woh"] + (["wol"] if WO_SPLIT else [])):
            t = wqop.tile([P, NCD, D], F8, tag=src, name=src)
            nc.sync.dma_start(t[:], dt[src][:])
            wo_sb.append(t)

        # ============== stage 1b: v projection + kv/ksum accumulation ========
        with ExitStack() as st1b:
            xvp = st1b.enter_context(tc.tile_pool(name="xv", bufs=2))
            g1vp = st1b.enter_context(tc.tile_pool(name="g1v", bufs=3))
            vgp = st1b.enter_context(tc.tile_pool(name="vgp", bufs=1))
            pv1 = st1b.enter_context(tc.tile_pool(name="pv1", bufs=3, space="PSUM"))
            pv2 = st1b.enter_context(tc.tile_pool(name="pv2", bufs=3, space="PSUM"))
            pkvp = st1b.enter_context(tc.tile_pool(name="pkv", bufs=1, space="PSUM"))

            vg_bufs = [vgp.tile([P, H, 65], BF16, tag=f"vg{i}", name=f"vg{i}")
                       for i in range(3)]
            psum_kv = [pkvp.tile([P, 260], F32, tag=f"pkv{i}", name=f"pkv{i}")
                       for i in range(2)]

            def kv_tail(m):
                vg = vg_bufs[m % 3]
                for h in range(H):
                    bank = psum_kv[h // 8]
                    pr = (h % 2) * 64
                    fc = ((h // 2) % 4) * 65
                    # HW start=True marks the WHOLE psum row (all columns) of
                    # the participating partitions pending-zero -- start only
                    # on the first head per (bank, partition-half); later
                    # heads' first writes overwrite via has_written
                    first = m == 0 and (h % 8) < 2
                    last = m == NM - 1 and (h % 8) >= 6
                    nc.tensor.matmul(
                        bank[pr:pr + 64, fc:fc + 65],
                        phi_k[m][:, h * DK:(h + 1) * DK],
                        vg[:, h:h + 1, :],
                        start=first, stop=last,
                        skip_group_check=not (first or last))

            for m in range(NM):
                xv_m = xvp.tile([P, NCD, P], F8, tag="xv", name="xv")
                nc.sync.dma_start(xv_m[:], dt["xv8"][m, :, :, :])
                vg = vg_bufs[m % 3]
                # ksum column: phi_k column of ones * mask (handles masking)
                nc.vector.tensor_scalar_mul(vg[:, :, 64:65], ones16[:],
                                            maskp_sb[:, m:m + 1])
                for half in range(2):
                    ns = slice(half * 512, (half + 1) * 512)
                    p1 = pv1.tile([P, 512], F32, tag="pv1", name="pv1")
                    p2 = pv2.tile([P, 512], F32, tag="pv2", name="pv2")
                    mm(p1[:], ones2_sb[:], bv_sb[0][:, :, ns], True, False)
                    for cp in range(4):
                        mm(p1[:], xv_m[:, 2 * cp:2 * cp + 2, :],
                           wv_sb["1h"][:, 2 * cp:2 * cp + 2, ns], False,
                           (not V_SPLIT) and cp == 3)
                    if V_SPLIT:
                        for cp in range(4):
                            mm(p1[:], xv_m[:, 2 * cp:2 * cp + 2, :],
                               wv_sb["1l"][:, 2 * cp:2 * cp + 2, ns], False,
                               cp == 3)
                    mm(p2[:], ones2_sb[:], bv_sb[1][:, :, ns], True, False)
                    for cp in range(4):
                        mm(p2[:], xv_m[:, 2 * cp:2 * cp + 2, :],
                           wv_sb["2h"][:, 2 * cp:2 * cp + 2, ns], False,
                           (not V_SPLIT) and cp == 3)
                    if V_SPLIT:
                        for cp in range(4):
                            mm(p2[:], xv_m[:, 2 * cp:2 * cp + 2, :],
                               wv_sb["2l"][:, 2 * cp:2 * cp + 2, ns], False,
                               cp == 3)
                    g1v = g1vp.tile([P, 512], BF16, tag="g1v", name="g1v")
                    act(g1v[:], p1[:], ACTF.Silu, scale=1.0 / 64)
                    # vg = (silu * mask) * p2  (64-scaled; mask folded here)
                    nc.vector.scalar_tensor_tensor(
                        vg[:, 8 * half:8 * half + 8, 0:64], g1v[:],
                        maskp_sb[:, m:m + 1], p2[:], ALU.mult, ALU.mult)
                if m >= 2:
                    kv_tail(m - 2)
            kv_tail(NM - 2)
            kv_tail(NM - 1)
            kvev = [kvstage.tile([P, 260], F32, tag=f"kvev{i}", name=f"kvev{i}")
                    for i in range(2)]
            for i in range(2):
                nc.vector.tensor_copy(kvev[i][:], psum_kv[i][:])

        stv.close()  # frees wv weights
        st1.close()  # frees phi_k SBUF before stage 2

        # ============ collective: pair AllReduce of kv+ksum ============
        cc_in = dram.tile([2 * P, 260], F32)
        cc_out = dram.tile([2 * P, 260], F32)
        nc.gpsimd.dma_start(cc_in[0:P, :], kvev[0][:])
        nc.gpsimd.dma_start(cc_in[P:2 * P, :], kvev[1][:])
        kvstage_ctx.close()
        if single_core:
            nc.gpsimd.dma_start(cc_out[:], cc_in[:])
        else:
            nc.gpsimd.collective_compute(
                "AllReduce", ALU.add, replica_groups=GROUPS,
                ins=[cc_in.opt()], outs=[cc_out.opt()])

        # repack: kv -> block-diag bf16 tiles; ksum -> block-diag bf16 tiles
        kv_bd = [kvres.tile([P, P], BF16, tag=f"kvbd{c}", name=f"kvbd{c}")
                 for c in range(NCD)]
        ksum_bd = [kvres.tile([P, H], BF16, tag=f"ksbd{c}", name=f"ksbd{c}")
                   for c in range(NCD)]
        with ExitStack() as strp:
            rpp = strp.enter_context(tc.tile_pool(name="rpp", bufs=1))
            kvf32 = rpp.tile([P, NCD, DK], F32, name="kvf32")
            ksf32 = rpp.tile([P, NCD], F32, name="ksf32")
            # cc_out(row=p [+128], col=cp*65+j) is affine in (p, cp, j):
            # head h=2cp+p//64 lives at row (h//8)*128+(h%2)*64+(p%64) = p [+128]
            base = cc_out
            for lo in range(2):
                off = lo * 4 * 65 * 0 + lo * P * 260  # high half: rows 128..255
                cps = slice(4 * lo, 4 * lo + 4)
                src_kv = bass.AP(base.tensor, base.offset + off,
                                 [[260, P], [65, 4], [1, DK]])
                nc.scalar.dma_start(kvf32[:, cps, :], src_kv)
                src_ks = bass.AP(base.tensor, base.offset + off + DK,
                                 [[260, P], [65, 4], [1, 1]])
                nc.scalar.dma_start(ksf32[:, cps], src_ks)
            for cp in range(NCD):
                nc.gpsimd.memset(kv_bd[cp][:], 0.0)
                nc.vector.tensor_copy(kv_bd[cp][0:64, 0:64],
                                      kvf32[0:64, cp:cp + 1, :])
                nc.vector.tensor_copy(kv_bd[cp][64:128, 64:128],
                                      kvf32[64:128, cp:cp + 1, :])
                nc.gpsimd.memset(ksum_bd[cp][:], 0.0)
                nc.vector.tensor_copy(ksum_bd[cp][0:64, 2 * cp:2 * cp + 1],
                                      ksf32[0:64, cp:cp + 1])
                nc.vector.tensor_copy(ksum_bd[cp][64:128, 2 * cp + 1:2 * cp + 2],
                                      ksf32[64:128, cp:cp + 1])

            # ============ stage 2: q -> phi_q -> z -> attn -> out ============
            with ExitStack() as st2:
                xqp = st2.enter_context(tc.tile_pool(name="xq", bufs=3))
                g1qp = st2.enter_context(tc.tile_pool(name="g1q", bufs=2))
                qgp = st2.enter_context(tc.tile_pool(name="qg", bufs=4))
                mint2 = st2.enter_context(tc.tile_pool(name="mint2", bufs=2))
                texp2 = st2.enter_context(tc.tile_pool(name="texp2", bufs=2))
                trel2 = st2.enter_context(tc.tile_pool(name="trel2", bufs=2))
                phiqp = st2.enter_context(tc.tile_pool(name="phiq", bufs=4))
                zepsp = st2.enter_context(tc.tile_pool(name="zeps", bufs=2))
                rsbp = st2.enter_context(tc.tile_pool(name="rsb", bufs=2))
                rrepp = st2.enter_context(tc.tile_pool(name="rrep", bufs=2))
                phiqrp = st2.enter_context(tc.tile_pool(name="phiqr", bufs=2))
                rdram = st2.enter_context(tc.tile_pool(name="rdram", bufs=2,
                                                       space="DRAM"))
                attnp = st2.enter_context(tc.tile_pool(name="attn", bufs=2))
                osbp = st2.enter_context(tc.tile_pool(name="osb", bufs=2))
                pq1 = st2.enter_context(tc.tile_pool(name="pq1", bufs=2,
                                                     space="PSUM"))
                pq2 = st2.enter_context(tc.tile_pool(name="pq2", bufs=2,
                                                     space="PSUM"))
                pzp = st2.enter_context(tc.tile_pool(name="pz", bufs=1,
                                                     space="PSUM"))
                pnp = st2.enter_context(tc.tile_pool(name="pn", bufs=2,
                                                     space="PSUM"))
                pop = st2.enter_context(tc.tile_pool(name="po", bufs=1,
                                                     space="PSUM"))

                xq_tiles = {}

                def get_xq(ch):
                    if ch not in xq_tiles:
                        t = xqp.tile([P, NCD, CH], F8, tag="xq", name="xq")
                        nc.sync.dma_start(t[:], dt["xq8"][ch, :, :, :])
                        xq_tiles[ch] = t
                    return xq_tiles[ch]

                def proj(ch):
                    xq_ch = get_xq(ch)
                    if ch + 2 < NCH:
                        get_xq(ch + 2)
                    qg = qgp.tile([P, NCD, CH], BF16, tag="qg", name="qg")
                    for mc in range(NCD):
                        ms = slice(mc * P, (mc + 1) * P)
                        p1 = pq1.tile([P, CH], F32, tag="pq1", name="pq1")
                        p2 = pq2.tile([P, CH], F32, tag="pq2", name="pq2")
                        for cp in range(4):
                            mm(p1[:], wq_sb[0][:, 2 * cp:2 * cp + 2, ms],
                               xq_ch[:, 2 * cp:2 * cp + 2, :], cp == 0, cp == 3)
                        for cp in range(4):
                            mm(p2[:], wq_sb[1][:, 2 * cp:2 * cp + 2, ms],
                               xq_ch[:, 2 * cp:2 * cp + 2, :], cp == 0, cp == 3)
                        g1 = g1qp.tile([P, CH], BF16, tag="g1q", name="g1q")
                        act(g1[:], p1[:], ACTF.Silu,
                            bias=bq1c_sb[:, mc:mc + 1], scale=1.0 / 64)
                        nc.vector.scalar_tensor_tensor(
                            qg[:, mc:mc + 1, :], p2[:],
                            bq2c64_sb[:, mc:mc + 1], g1[:], ALU.add, ALU.mult)
                    return qg

                def phiq_build(qg):
                    mint = mint2.tile([P, NCD, CH], BF16, tag="mintq",
                                      name="mintq")
                    nc.vector.tensor_scalar_min(mint[:], qg[:], 0.0)
                    texp = texp2.tile([P, NCD, CH], BF16, tag="texpq",
                                      name="texpq")
                    act(texp[:], mint[:], ACTF.Exp, scale=1.0 / 64)
                    trel = trel2.tile([P, NCD, CH], BF16, tag="trelq",
                                      name="trelq")
                    nc.vector.tensor_scalar(trel[:], qg[:], 0.0, 1.0 / 64,
                                            ALU.max, ALU.mult)
                    phiq = phiqp.tile([P, NCD, CH], BF16, tag="phiq",
                                      name="phiq")
                    nc.vector.tensor_tensor(phiq[:], texp[:], trel[:], ALU.add)
                    return phiq

                def rhead(ch, phiq):
                    pz = pzp.tile([H, CH], F32, tag="pz", name="pz")[:]
                    for cp in range(NCD):
                        nc.tensor.matmul(pz, ksum_bd[cp][:],
                                         phiq[:, cp:cp + 1, :],
                                         start=cp == 0, stop=cp == NCD - 1)
                    zeps = zepsp.tile([H, CH], F32, tag="zeps", name="zeps")
                    nc.vector.tensor_scalar_add(zeps[:], pz, EPS)
                    rsb16 = rsbp.tile([H, CH], BF16, tag="rsb16", name="rsb16")
                    with nc.allow_low_precision(reason="r is consumed in bf16"):
                        nc.vector.reciprocal(rsb16[:], zeps[:])
                    # broadcast r across partitions via a DRAM round-trip: a
                    # stride-0 partition AP replicates row 2cp(+1) to 64 rows;
                    # launched a full chunk-pair before its consumers so the
                    # ~4us round trip never blocks the PE
                    rd = rdram.tile([H, CH], BF16, tag="rd", name="rd")
                    nc.sync.dma_start(rd[:], rsb16[:])
                    rrep = rrepp.tile([P, NCD, CH], BF16, tag="rrep",
                                      name="rrep")
                    base = rd[:]
                    src_lo = bass.AP(base.tensor, base.offset,
                                     [[0, 64], [2 * CH, NCD], [1, CH]])
                    src_hi = bass.AP(base.tensor, base.offset + CH,
                                     [[0, 64], [2 * CH, NCD], [1, CH]])
                    nc.sync.dma_start(rrep[0:64, :, :], src_lo)
                    nc.sync.dma_start(rrep[64:128, :, :], src_hi)
                    return rrep

                def tail2(ch, phiq, rrep):
                    # pre-scale phi_q by 1/z in ONE bf16 4x-mode DVE op, so the
                    # num matmul emits 64*attn directly and psum evicts are copies
                    phiqr = phiqrp.tile([P, NCD, CH], BF16, tag="phiqr",
                                        name="phiqr")
                    nc.vector.tensor_tensor(phiqr[:], phiq[:], rrep[:], ALU.mult)
                    attn = attnp.tile([P, NCD, CH], F8, tag="attn", name="attn")
                    for cp in range(NCD):
                        pn = pnp.tile([P, CH], F32, tag="pn", name="pn")
                        nc.tensor.matmul(pn[:], kv_bd[cp][:],
                                         phiqr[:, cp:cp + 1, :],
                                         start=True, stop=True)
                        if cp % 2 == 0:
                            nc.scalar.activation(attn[:, cp:cp + 1, :], pn[:],
                                                 ACTF.Copy)
                        else:
                            nc.vector.tensor_copy(attn[:, cp:cp + 1, :], pn[:])
                    for mt in range(CH // P):
                        for nh in range(2):
                            ns = slice(nh * 512, (nh + 1) * 512)
                            po = pop.tile([P, 512], F32, tag="po", name="po")
                            nmm = len(wo_sb) * 4
                            i = 0
                            for wo_t in wo_sb:
                                for cp in range(4):
                                    mm(po[:],
                                       attn[:, 2 * cp:2 * cp + 2,
                                            mt * P:(mt + 1) * P],
                                       wo_t[:, 2 * cp:2 * cp + 2, ns],
                                       i == 0, i == nmm - 1)
                                    i += 1
                            osb = osbp.tile([P, 512], F32, tag="osb", name="osb")
                            if (mt + nh) % 2 == 0:
                                nc.scalar.activation(osb[:], po[:], ACTF.Copy)
                            else:
                                nc.vector.tensor_copy(osb[:], po[:])
                            row0 = ch * CH + mt * P
                            nc.gpsimd.dma_start(out[row0:row0 + P, ns], osb[:])

                get_xq(0)
                get_xq(1)
                pending = None
                for pr2 in range(NCH // 2):
                    qgs = [proj(2 * pr2), proj(2 * pr2 + 1)]
                    if pending is not None:
                        for c2, ph2, rr2 in pending:
                            tail2(c2, ph2, rr2)
                    phis = [phiq_build(qgs[i]) for i in range(2)]
                    pending = [(2 * pr2 + i, phis[i],
                                rhead(2 * pr2 + i, phis[i])) for i in range(2)]
                for c2, ph2, rr2 in pending:
                    tail2(c2, ph2, rr2)


# revision 29
# speedup vs baseline: 1.0073x; 1.0073x over previous
"""MultiHeadLinearAttention Trainium2 kernel (8-core SPMD, fp8 DoubleRow).

Sharding: 16384 tokens split across 8 cores (core c: batch c//2, sequence half
c%2). All projections/attention/out-proj are local; the only cross-core
dependency is the per-batch KV summary (kv+ksum packed [256,260]) reduced via a
pair-wise AllReduce.

Numerics (validated in numpy sim; rel err ~1e-2 vs 2e-2 gate):
  - x and all weights pre-quantized host-side to fp8e4m3; weights scaled x64
    (uniform(-1/32,1/32) would be subnormal in fp8 otherwise)
  - all 7 projections run as fp8 DoubleRow matmuls (K=256/instr, 0.5 cyc/row)
  - q,k,v GLUs single-pass fp8 (q/k errors cancel in the num/z ratio);
    the out projection uses weight-split error feedback (W ~ W_hi + W_lo)
  - GLU intermediates bf16 (DVE 2x/4x modes need 2-byte dtypes);
    kv/z/num matmuls bf16; r kept fp32; attn stored as fp8(64*attn)
  - biases enter PSUM via K=1 DoubleRow matmuls (token-major k/v) or ACT
    bias (feature-major q); out bias + 1/4096 unscale applied on host

Engine balance: PE ~170us is the roof; elementwise split DVE/ACT/Pool:
  - ACT: silu/exp only (exp batched by groups of 8 tiles / chunk pairs --
    silu and exp live in different act tables, each switch costs 1283ns)
  - DVE: GLU products (psum reads), phi assembly (bf16 ts/tt fast modes),
    attn eviction, half the out evictions
  - Pool/gpsimd: 1/z partition-broadcast (replaces PE broadcast matmuls),
    weight DMA issue, collective
"""
import os
from contextlib import ExitStack

import ml_dtypes
import numpy as np
import bass_rust
import concourse.bass as bass
import concourse.mybir as mybir
import concourse.tile as tile
from concourse import bacc
from concourse.bass_utils import run_bass_kernel_spmd

F32 = mybir.dt.float32
BF16 = mybir.dt.bfloat16
F8 = mybir.dt.float8e4
ACTF = mybir.ActivationFunctionType
ALU = mybir.AluOpType
DR = mybir.MatmulPerfMode.DoubleRow

B, S, D, H = 4, 4096, 1024, 16
DK = D // H          # 64
EPS = 1e-6
NCORES = 8
T = B * S // NCORES  # 2048 tokens per core
P = 128
NM = T // P          # 16 token tiles
NCD = D // P         # 8 d-chunks
CH = 256             # stage-2 token chunk
NCH = T // CH        # 8 chunks
GB = 8               # stage-1a exp batch group size
GROUPS = [[0, 1], [2, 3], [4, 5], [6, 7]]

V_SPLIT = False      # single-pass fp8 v GLU (stage-1 is PE-bound; error ~11.4e-3)
WO_SPLIT = True      # weight-split error feedback on the out projection

NP8 = ml_dtypes.float8_e4m3


def build(single_core=False):
    nc = bacc.Bacc("TRN2", target_bir_lowering=False, debug=False,
                   num_devices=1 if single_core else NCORES)
    dt_in = {}

    def inp(name, shape, dtype=F8):
        dt_in[name] = nc.dram_tensor(name, shape, dtype, kind="ExternalInput").ap()

    inp("xk8", [NM, P, NCD, P])
    inp("xv8", [NM, P, NCD, P])
    inp("xq8", [NCH, P, NCD, CH])
    for nm in ("wk1", "wk2", "wq1", "wq2"):
        inp(nm, [P, NCD, D])
    vws = ["wv1h", "wv2h"] + (["wv1l", "wv2l"] if V_SPLIT else [])
    for nm in vws:
        inp(nm, [P, NCD, D])
    ows = ["woh"] + (["wol"] if WO_SPLIT else [])
    for nm in ows:
        inp(nm, [P, NCD, D])
    for nm in ("bk1", "bk2", "bv1", "bv2"):
        inp(nm, [1, 2, D])
    inp("ones2", [1, 2, P])
    inp("bq1c", [P, NCD], F32)
    inp("bq2c64", [P, NCD], F32)
    inp("maskp", [P, NM], F32)
    out = nc.dram_tensor("out", [T, D], F32, kind="ExternalOutput").ap()

    with tile.TileContext(nc) as tc:
        _emit(nc, tc, dt_in, out, single_core)
    nc.compile()
    return nc


def _emit(nc, tc, dt, out, single_core):
    def mm(ps, l, r, start, stop, skip=False):
        nc.tensor.matmul(ps, l, r, start=start, stop=stop, perf_mode=DR,
                         skip_group_check=skip)

    # chain ACT instructions so the scheduler preserves emission order --
    # silu and exp live in different act-func tables (1283ns per switch), and
    # the list scheduler otherwise interleaves them freely
    last_act = [None]

    def act(out_ap, in_ap, func, **kw):
        inst = nc.scalar.activation(out_ap, in_ap, func, **kw)
        if last_act[0] is not None:
            bass_rust.add_dep_helper(inst.ins, last_act[0].ins,
                                     reason="act-table-order")
        last_act[0] = inst
        return inst

    with ExitStack() as st0:
        const = st0.enter_context(tc.tile_pool(name="const", bufs=1))
        dram = st0.enter_context(tc.tile_pool(name="dram", bufs=1, space="DRAM"))
        kvres = st0.enter_context(tc.tile_pool(name="kvres", bufs=1))

        ones2_sb = const.tile([1, 2, P], F8, name="ones2_sb")
        nc.scalar.dma_start(ones2_sb[:], dt["ones2"][:])
        bq1c_sb = const.tile([P, NCD], F32, name="bq1c_sb")
        nc.scalar.dma_start(bq1c_sb[:], dt["bq1c"][:])
        bq2c64_sb = const.tile([P, NCD], F32, name="bq2c64_sb")
        nc.scalar.dma_start(bq2c64_sb[:], dt["bq2c64"][:])
        maskp_sb = const.tile([P, NM], F32, name="maskp_sb")
        nc.scalar.dma_start(maskp_sb[:], dt["maskp"][:])
        ones16 = const.tile([P, H], BF16, name="ones16")
        nc.gpsimd.memset(ones16[:], 1.0)

        # stage-2 weight pool at st0 scope so DMAs can prefetch during stage 1b
        wqop = st0.enter_context(tc.tile_pool(name="wqop", bufs=1, side="right"))

        kvstage = st0.enter_context(tc.tile_pool(name="kvstage", bufs=1))

        st1 = st0.enter_context(ExitStack())
        phik_pool = st1.enter_context(tc.tile_pool(name="phik", bufs=1))
        phi_k = [phik_pool.tile([P, D], BF16, tag=f"phik_{m}", name=f"phik_{m}")
                 for m in range(NM)]

        # wv pool spans stage 1a (prefetch) + stage 1b (use)
        stv = ExitStack()
        wvp = stv.enter_context(tc.tile_pool(name="wv", bufs=1))

        # ================= stage 1a: k projection -> phi_k =================
        with ExitStack() as st1a:
            wkp = st1a.enter_context(tc.tile_pool(name="wk", bufs=1))
            xkp = st1a.enter_context(tc.tile_pool(name="xk", bufs=2))
            g1p = st1a.enter_context(tc.tile_pool(name="g1p", bufs=2))
            kgp = st1a.enter_context(tc.tile_pool(name="kgp", bufs=2))
            mintp = st1a.enter_context(tc.tile_pool(name="mintp", bufs=GB + 2))
            trelp = st1a.enter_context(tc.tile_pool(name="trelp", bufs=GB + 2))
            texpp = st1a.enter_context(tc.tile_pool(name="texpp", bufs=2))
            pk1 = st1a.enter_context(tc.tile_pool(name="pk1", bufs=4, space="PSUM"))
            pk2 = st1a.enter_context(tc.tile_pool(name="pk2", bufs=4, space="PSUM"))

            bk_sb = {}
            for w, src in ((0, "bk1"), (1, "bk2")):
                bk_sb[w] = wkp.tile([1, 2, D], F8, tag=f"bk{w}", name=f"bk{w}")
                nc.sync.dma_start(bk_sb[w][:], dt[src][:])
            xk0 = xkp.tile([P, NCD, P], F8, tag="xk", name="xk0")
            nc.sync.dma_start(xk0[:], dt["xk8"][0, :, :, :])
            wk_sb = {}
            for w, src in ((0, "wk1"), (1, "wk2")):
                wk_sb[w] = wkp.tile([P, NCD, D], F8, tag=f"wk{w}", name=f"wk{w}")
                for hf in range(2):
                    ns = slice(hf * 512, (hf + 1) * 512)
                    nc.sync.dma_start(wk_sb[w][:, :, ns], dt[src][:, :, ns])
            wv_sb = {}
            bv_sb = {}

            def wv_prefetch():
                vnames = [("1h", "wv1h"), ("2h", "wv2h")]
                if V_SPLIT:
                    vnames += [("1l", "wv1l"), ("2l", "wv2l")]
                for w, src in vnames:
                    wv_sb[w] = wvp.tile([P, NCD, D], F8, tag=f"wv{w}",
                                        name=f"wv{w}")
                    nc.scalar.dma_start(wv_sb[w][:], dt[src][:])
                for w, src in ((0, "bv1"), (1, "bv2")):
                    bv_sb[w] = wvp.tile([1, 2, D], F8, tag=f"bv{w}",
                                        name=f"bv{w}")
                    nc.scalar.dma_start(bv_sb[w][:], dt[src][:])

            mints, trels = {}, {}

            def phi_flush(ms):
                for m2 in ms:
                    texp = texpp.tile([P, D], BF16, tag="texp", name="texp")
                    act(texp[:], mints[m2][:], ACTF.Exp, scale=1.0 / 64)
                    nc.vector.tensor_tensor(phi_k[m2][:], texp[:], trels[m2][:],
                                            ALU.add)

            for m in range(NM):
                xk_m = xkp.tile([P, NCD, P], F8, tag="xk", name="xk")
                nc.sync.dma_start(xk_m[:], dt["xk8"][m, :, :, :])
                kg = kgp.tile([P, D], BF16, tag="kg", name="kg")
                g1 = g1p.tile([P, D], BF16, tag="g1", name="g1")
                for half in range(2):
                    ns = slice(half * 512, (half + 1) * 512)
                    p1 = pk1.tile([P, 512], F32, tag="p1", name="p1")
                    p2 = pk2.tile([P, 512], F32, tag="p2", name="p2")
                    mm(p1[:], ones2_sb[:], bk_sb[0][:, :, ns], True, False)
                    for cp in range(4):
                        mm(p1[:], xk_m[:, 2 * cp:2 * cp + 2, :],
                           wk_sb[0][:, 2 * cp:2 * cp + 2, ns], False, cp == 3)
                    mm(p2[:], ones2_sb[:], bk_sb[1][:, :, ns], True, False)
                    for cp in range(4):
                        mm(p2[:], xk_m[:, 2 * cp:2 * cp + 2, :],
                           wk_sb[1][:, 2 * cp:2 * cp + 2, ns], False, cp == 3)
                    act(g1[:, ns], p1[:], ACTF.Silu, scale=1.0 / 64)
                    nc.vector.tensor_tensor(kg[:, ns], g1[:, ns], p2[:], ALU.mult)
                mint = mintp.tile([P, D], BF16, tag="mint", name="mint")
                nc.vector.tensor_scalar_min(mint[:], kg[:], 0.0)
                trel = trelp.tile([P, D], BF16, tag="trel", name="trel")
                nc.vector.tensor_scalar(trel[:], kg[:], 0.0, 1.0 / 64,
                                        ALU.max, ALU.mult)
                mints[m], trels[m] = mint, trel
                if m == 1:
                    wv_prefetch()
                if m % GB == GB - 1:
                    phi_flush(range(m - GB + 1, m + 1))

        # prefetch stage-2 weights during stage 1b
        wq_sb = {}
        for w, src in ((0, "wq1"), (1, "wq2")):
            wq_sb[w] = wqop.tile([P, NCD, D], F8, tag=f"wq{w}", name=f"wq{w}")
            nc.sync.dma_start(wq_sb[w][:], dt[src][:])
        wo_sb = []
        for src in (["woh"] + (["wol"] if WO_SPLIT else [])):
            t = wqop.tile([P, NCD, D], F8, tag=src, name=src)
            nc.sync.dma_start(t[:], dt[src][:])
            wo_sb.append(t)

        # ============== stage 1b: v projection + kv/ksum accumulation ========
        with ExitStack() as st1b:
            xvp = st1b.enter_context(tc.tile_pool(name="xv", bufs=2))
            g1vp = st1b.enter_context(tc.tile_pool(name="g1v", bufs=3))
            vgp = st1b.enter_context(tc.tile_pool(name="vgp", bufs=1))
            pv1 = st1b.enter_context(tc.tile_pool(name="pv1", bufs=3, space="PSUM"))
            pv2 = st1b.enter_context(tc.tile_pool(name="pv2", bufs=3, space="PSUM"))
            pkvp = st1b.enter_context(tc.tile_pool(name="pkv", bufs=1, space="PSUM"))

            vg_bufs = [vgp.tile([P, H, 65], BF16, tag=f"vg{i}", name=f"vg{i}")
                       for i in range(3)]
            psum_kv = [pkvp.tile([P, 260], F32, tag=f"pkv{i}", name=f"pkv{i}")
                       for i in range(2)]

            def kv_tail(m):
                vg = vg_bufs[m % 3]
                for h in range(H):
                    bank = psum_kv[h // 8]
                    pr = (h % 2) * 64
                    fc = ((h // 2) % 4) * 65
                    # HW start=True marks the WHOLE psum row (all columns) of
                    # the participating partitions pending-zero -- start only
                    # on the first head per (bank, partition-half); later
                    # heads' first writes overwrite via has_written
                    first = m == 0 and (h % 8) < 2
                    last = m == NM - 1 and (h % 8) >= 6
                    nc.tensor.matmul(
                        bank[pr:pr + 64, fc:fc + 65],
                        phi_k[m][:, h * DK:(h + 1) * DK],
                        vg[:, h:h + 1, :],
                        start=first, stop=last,
                        skip_group_check=not (first or last))

            for m in range(NM):
                xv_m = xvp.tile([P, NCD, P], F8, tag="xv", name="xv")
                nc.sync.dma_start(xv_m[:], dt["xv8"][m, :, :, :])
                vg = vg_bufs[m % 3]
                # ksum column: phi_k column of ones * mask (handles masking)
                nc.vector.tensor_scalar_mul(vg[:, :, 64:65], ones16[:],
                                            maskp_sb[:, m:m + 1])
                for half in range(2):
                    ns = slice(half * 512, (half + 1) * 512)
                    p1 = pv1.tile([P, 512], F32, tag="pv1", name="pv1")
                    p2 = pv2.tile([P, 512], F32, tag="pv2", name="pv2")
                    mm(p1[:], ones2_sb[:], bv_sb[0][:, :, ns], True, False)
                    for cp in range(4):
                        mm(p1[:], xv_m[:, 2 * cp:2 * cp + 2, :],
                           wv_sb["1h"][:, 2 * cp:2 * cp + 2, ns], False,
                           (not V_SPLIT) and cp == 3)
                    if V_SPLIT:
                        for cp in range(4):
                            mm(p1[:], xv_m[:, 2 * cp:2 * cp + 2, :],
                               wv_sb["1l"][:, 2 * cp:2 * cp + 2, ns], False,
                               cp == 3)
                    mm(p2[:], ones2_sb[:], bv_sb[1][:, :, ns], True, False)
                    for cp in range(4):
                        mm(p2[:], xv_m[:, 2 * cp:2 * cp + 2, :],
                           wv_sb["2h"][:, 2 * cp:2 * cp + 2, ns], False,
                           (not V_SPLIT) and cp == 3)
                    if V_SPLIT:
                        for cp in range(4):
                            mm(p2[:], xv_m[:, 2 * cp:2 * cp + 2, :],
                               wv_sb["2l"][:, 2 * cp:2 * cp + 2, ns], False,
                               cp == 3)
                    g1v = g1vp.tile([P, 512], BF16, tag="g1v", name="g1v")
                    act(g1v[:], p1[:], ACTF.Silu, scale=1.0 / 64)
                    # vg = (silu * mask) * p2  (64-scaled; mask folded here)
                    nc.vector.scalar_tensor_tensor(
                        vg[:, 8 * half:8 * half + 8, 0:64], g1v[:],
                        maskp_sb[:, m:m + 1], p2[:], ALU.mult, ALU.mult)
                if m >= 2:
                    kv_tail(m - 2)
            kv_tail(NM - 2)
            kv_tail(NM - 1)
            kvev = [kvstage.tile([P, 260], F32, tag=f"kvev{i}", name=f"kvev{i}")
                    for i in range(2)]
            for i in range(2):
                nc.vector.tensor_copy(kvev[i][:], psum_kv[i][:])

        stv.close()  # frees wv weights
        st1.close()  # frees phi_k SBUF before stage 2

        # ============ collective: pair AllReduce of kv+ksum ============
        cc_in = dram.tile([2 * P, 260], F32)
        cc_out = dram.tile([2 * P, 260], F32)
        nc.gpsimd.dma_start(cc_in[0:P, :], kvev[0][:])
        nc.gpsimd.dma_start(cc_in[P:2 * P, :], kvev[1][:])
        kvstage_ctx.close()
        if single_core:
            nc.gpsimd.dma_start(cc_out[:], cc_in[:])
        else:
            nc.gpsimd.collective_compute(
                "AllReduce", ALU.add, replica_groups=GROUPS,
                ins=[cc_in.opt()], outs=[cc_out.opt()])

        # repack: kv -> block-diag bf16 tiles; ksum -> block-diag bf16 tiles
        kv_bd = [kvres.tile([P, P], BF16, tag=f"kvbd{c}", name=f"kvbd{c}")
                 for c in range(NCD)]
        ksum_bd = [kvres.tile([P, H], BF16, tag=f"ksbd{c}", name=f"ksbd{c}")
                   for c in range(NCD)]
        with ExitStack() as strp:
            rpp = strp.enter_context(tc.tile_pool(name="rpp", bufs=1))
            kvf32 = rpp.tile([P, NCD, DK], F32, name="kvf32")
            ksf32 = rpp.tile([P, NCD], F32, name="ksf32")
            # cc_out(row=p [+128], col=cp*65+j) is affine in (p, cp, j):
            # head h=2cp+p//64 lives at row (h//8)*128+(h%2)*64+(p%64) = p [+128]
            base = cc_out
            for lo in range(2):
                off = lo * 4 * 65 * 0 + lo * P * 260  # high half: rows 128..255
                cps = slice(4 * lo, 4 * lo + 4)
                src_kv = bass.AP(base.tensor, base.offset + off,
                                 [[260, P], [65, 4], [1, DK]])
                nc.scalar.dma_start(kvf32[:, cps, :], src_kv)
                src_ks = bass.AP(base.tensor, base.offset + off + DK,
                                 [[260, P], [65, 4], [1, 1]])
                nc.scalar.dma_start(ksf32[:, cps], src_ks)
            for cp in range(NCD):
                nc.gpsimd.memset(kv_bd[cp][:], 0.0)
                nc.vector.tensor_copy(kv_bd[cp][0:64, 0:64],
                                      kvf32[0:64, cp:cp + 1, :])
                nc.vector.tensor_copy(kv_bd[cp][64:128, 64:128],
                                      kvf32[64:128, cp:cp + 1, :])
                nc.gpsimd.memset(ksum_bd[cp][:], 0.0)
                nc.vector.tensor_copy(ksum_bd[cp][0:64, 2 * cp:2 * cp + 1],
                                      ksf32[0:64, cp:cp + 1])
                nc.vector.tensor_copy(ksum_bd[cp][64:128, 2 * cp + 1:2 * cp + 2],
                                      ksf32[64:128, cp:cp + 1])

            # ============ stage 2: q -> phi_q -> z -> attn -> out ============
            with ExitStack() as st2:
                xqp = st2.enter_context(tc.tile_pool(name="xq", bufs=3))
                g1qp = st2.enter_context(tc.tile_pool(name="g1q", bufs=2))
                qgp = st2.enter_context(tc.tile_pool(name="qg", bufs=4))
                mint2 = st2.enter_context(tc.tile_pool(name="mint2", bufs=2))
                texp2 = st2.enter_context(tc.tile_pool(name="texp2", bufs=2))
                trel2 = st2.enter_context(tc.tile_pool(name="trel2", bufs=2))
                phiqp = st2.enter_context(tc.tile_pool(name="phiq", bufs=4))
                zepsp = st2.enter_context(tc.tile_pool(name="zeps", bufs=2))
                rsbp = st2.enter_context(tc.tile_pool(name="rsb", bufs=2))
                rrepp = st2.enter_context(tc.tile_pool(name="rrep", bufs=2))
                phiqrp = st2.enter_context(tc.tile_pool(name="phiqr", bufs=2))
                rdram = st2.enter_context(tc.tile_pool(name="rdram", bufs=2,
                                                       space="DRAM"))
                attnp = st2.enter_context(tc.tile_pool(name="attn", bufs=2))
                osbp = st2.enter_context(tc.tile_pool(name="osb", bufs=2))
                pq1 = st2.enter_context(tc.tile_pool(name="pq1", bufs=2,
                                                     space="PSUM"))
                pq2 = st2.enter_context(tc.tile_pool(name="pq2", bufs=2,
                                                     space="PSUM"))
                pzp = st2.enter_context(tc.tile_pool(name="pz", bufs=1,
                                                     space="PSUM"))
                pnp = st2.enter_context(tc.tile_pool(name="pn", bufs=2,
                                                     space="PSUM"))
                pop = st2.enter_context(tc.tile_pool(name="po", bufs=1,
                                                     space="PSUM"))

                xq_tiles = {}

                def get_xq(ch):
                    if ch not in xq_tiles:
                        t = xqp.tile([P, NCD, CH], F8, tag="xq", name="xq")
                        nc.sync.dma_start(t[:], dt["xq8"][ch, :, :, :])
                        xq_tiles[ch] = t
                    return xq_tiles[ch]

                def proj(ch):
                    xq_ch = get_xq(ch)
                    if ch + 2 < NCH:
                        get_xq(ch + 2)
                    qg = qgp.tile([P, NCD, CH], BF16, tag="qg", name="qg")
                    for mc in range(NCD):
                        ms = slice(mc * P, (mc + 1) * P)
                        p1 = pq1.tile([P, CH], F32, tag="pq1", name="pq1")
                        p2 = pq2.tile([P, CH], F32, tag="pq2", name="pq2")
                        for cp in range(4):
                            mm(p1[:], wq_sb[0][:, 2 * cp:2 * cp + 2, ms],
                               xq_ch[:, 2 * cp:2 * cp + 2, :], cp == 0, cp == 3)
                        for cp in range(4):
                            mm(p2[:], wq_sb[1][:, 2 * cp:2 * cp + 2, ms],
                               xq_ch[:, 2 * cp:2 * cp + 2, :], cp == 0, cp == 3)
                        g1 = g1qp.tile([P, CH], BF16, tag="g1q", name="g1q")
                        act(g1[:], p1[:], ACTF.Silu,
                            bias=bq1c_sb[:, mc:mc + 1], scale=1.0 / 64)
                        nc.vector.scalar_tensor_tensor(
                            qg[:, mc:mc + 1, :], p2[:],
                            bq2c64_sb[:, mc:mc + 1], g1[:], ALU.add, ALU.mult)
                    return qg

                def phiq_build(qg):
                    mint = mint2.tile([P, NCD, CH], BF16, tag="mintq",
                                      name="mintq")
                    nc.vector.tensor_scalar_min(mint[:], qg[:], 0.0)
                    texp = texp2.tile([P, NCD, CH], BF16, tag="texpq",
                                      name="texpq")
                    act(texp[:], mint[:], ACTF.Exp, scale=1.0 / 64)
                    trel = trel2.tile([P, NCD, CH], BF16, tag="trelq",
                                      name="trelq")
                    nc.vector.tensor_scalar(trel[:], qg[:], 0.0, 1.0 / 64,
                                            ALU.max, ALU.mult)
                    phiq = phiqp.tile([P, NCD, CH], BF16, tag="phiq",
                                      name="phiq")
                    nc.vector.tensor_tensor(phiq[:], texp[:], trel[:], ALU.add)
                    return phiq

                def rhead(ch, phiq):
                    pz = pzp.tile([H, CH], F32, tag="pz", name="pz")[:]
                    for cp in range(NCD):
                        nc.tensor.matmul(pz, ksum_bd[cp][:],
                                         phiq[:, cp:cp + 1, :],
                                         start=cp == 0, stop=cp == NCD - 1)
                    zeps = zepsp.tile([H, CH], F32, tag="zeps", name="zeps")
                    nc.vector.tensor_scalar_add(zeps[:], pz, EPS)
                    rsb16 = rsbp.tile([H, CH], BF16, tag="rsb16", name="rsb16")
                    with nc.allow_low_precision(reason="r is consumed in bf16"):
                        nc.vector.reciprocal(rsb16[:], zeps[:])
                    # broadcast r across partitions via a DRAM round-trip: a
                    # stride-0 partition AP replicates row 2cp(+1) to 64 rows;
                    # launched a full chunk-pair before its consumers so the
                    # ~4us round trip never blocks the PE
                    rd = rdram.tile([H, CH], BF16, tag="rd", name="rd")
                    nc.sync.dma_start(rd[:], rsb16[:])
                    rrep = rrepp.tile([P, NCD, CH], BF16, tag="rrep",
                                      name="rrep")
                    base = rd[:]
                    src_lo = bass.AP(base.tensor, base.offset,
                                     [[0, 64], [2 * CH, NCD], [1, CH]])
                    src_hi = bass.AP(base.tensor, base.offset + CH,
                                     [[0, 64], [2 * CH, NCD], [1, CH]])
                    nc.sync.dma_start(rrep[0:64, :, :], src_lo)
                    nc.sync.dma_start(rrep[64:128, :, :], src_hi)
                    return rrep

                def tail2(ch, phiq, rrep):
                    # pre-scale phi_q by 1/z in ONE bf16 4x-mode DVE op, so the
                    # num matmul emits 64*attn directly and psum evicts are copies
                    phiqr = phiqrp.tile([P, NCD, CH], BF16, tag="phiqr",
                                        name="phiqr")
                    nc.vector.tensor_tensor(phiqr[:], phiq[:], rrep[:], ALU.mult)
                    attn = attnp.tile([P, NCD, CH], F8, tag="attn", name="attn")
                    for cp in range(NCD):
                        pn = pnp.tile([P, CH], F32, tag="pn", name="pn")
                        nc.tensor.matmul(pn[:], kv_bd[cp][:],
                                         phiqr[:, cp:cp + 1, :],
                                         start=True, stop=True)
                        if cp % 2 == 0:
                            nc.scalar.activation(attn[:, cp:cp + 1, :], pn[:],
                                                 ACTF.Copy)
                        else:
                            nc.vector.tensor_copy(attn[:, cp:cp + 1, :], pn[:])
                    for mt in range(CH // P):
                        for nh in range(2):
                            ns = slice(nh * 512, (nh + 1) * 512)
                            po = pop.tile([P, 512], F32, tag="po", name="po")
                            nmm = len(wo_sb) * 4
                            i = 0
                            for wo_t in wo_sb:
                                for cp in range(4):
                                    mm(po[:],
                                       attn[:, 2 * cp:2 * cp + 2,
                                            mt * P:(mt + 1) * P],
                                       wo_t[:, 2 * cp:2 * cp + 2, ns],
                                       i == 0, i == nmm - 1)
                                    i += 1
                            osb = osbp.tile([P, 512], F32, tag="osb", name="osb")
                            if (mt + nh) % 2 == 0:
                                nc.scalar.activation(osb[:], po[:], ACTF.Copy)
                            else:
                                nc.vector.tensor_copy(osb[:], po[:])
                            row0 = ch * CH + mt * P
                            if ch >= NCH - 2:
                                # tail chunks: HWDGE (parallel queues), not the
                                # serial Pool SWDGE -- shortens the end drain
                                nc.sync.dma_start(out[row0:row0 + P, ns], osb[:])
                            else:
                                nc.gpsimd.dma_start(out[row0:row0 + P, ns],
                                                    osb[:])

                get_xq(0)
                get_xq(1)
                pending = None
                for pr2 in range(NCH // 2):
                    qgs = [proj(2 * pr2), proj(2 * pr2 + 1)]
                    if pending is not None:
                        for c2, ph2, rr2 in pending:
                            tail2(c2, ph2, rr2)
                    phis = [phiq_build(qgs[i]) for i in range(2)]
                    pending = [(2 * pr2 + i, phis[i],
                                rhead(2 * pr2 + i, phis[i])) for i in range(2)]
                for c2, ph2, rr2 in pending:
                    tail2(c2, ph2, rr2)
